# revision 1
# baseline (speedup 1.0000x reference)
import sys

if "/opt/trn_rl_repo" not in sys.path:
    sys.path.insert(0, "/opt/trn_rl_repo")

import numpy as np
import ml_dtypes

DIM = 1024
E = 8
H = 4096
T = 4096
NCORES = 8
P = 128
DKO = DIM // P
HKO = H // P
SH = H // NCORES
SHKO = SH // P
SLICE = 512
NSLICES = T // SLICE

BF16 = ml_dtypes.bfloat16

_nc_cache = {}


def _build_nc(sim=False):
    import concourse.mybir as mybir
    import concourse.tile as tile
    from concourse import bacc
    from concourse.masks import make_identity

    f32 = mybir.dt.float32
    bf16 = mybir.dt.bfloat16
    AF = mybir.ActivationFunctionType
    OP = mybir.AluOpType
    AX = mybir.AxisListType

    ndev = 1 if sim else NCORES
    nc = bacc.Bacc("TRN2", target_bir_lowering=False, debug=False, num_devices=ndev)

    xtbf = nc.dram_tensor("xtbf", [P, DKO, T], bf16, kind="ExternalInput")
    xt32 = nc.dram_tensor("xt32", [P, DKO, T], f32, kind="ExternalInput")
    rwp = nc.dram_tensor("rwp", [P, DKO, E], f32, kind="ExternalInput")
    rb = nc.dram_tensor("rb", [P, E], f32, kind="ExternalInput")
    w1p = nc.dram_tensor("w1p", [P, DKO, H], bf16, kind="ExternalInput")
    b1c = nc.dram_tensor("b1c", [P, HKO], f32, kind="ExternalInput")
    w2b = nc.dram_tensor("w2b", [DKO, P, HKO, P], bf16, kind="ExternalInput")
    b2c = nc.dram_tensor("b2c", [P, DKO], f32, kind="ExternalInput")
    sw1p = nc.dram_tensor("sw1p", [P, DKO, SH], bf16, kind="ExternalInput")
    sb1c = nc.dram_tensor("sb1c", [P, SHKO], f32, kind="ExternalInput")
    sw2p = nc.dram_tensor("sw2p", [P, SHKO, DIM], bf16, kind="ExternalInput")
    sb2c = nc.dram_tensor("sb2c", [P, DKO], f32, kind="ExternalInput")
    oh = nc.dram_tensor("oh", [E, P], f32, kind="ExternalInput")
    out = nc.dram_tensor("out", [P, T], f32, kind="ExternalOutput")

    with tile.TileContext(nc) as tc:
        with (
            tc.tile_pool(name="const", bufs=1) as const,
            tc.tile_pool(name="wpool", bufs=1) as wpool,
        ):
            ident = const.tile([P, P], f32)
            make_identity(nc, ident)
            rwp_sb = const.tile([P, DKO, E], f32)
            nc.sync.dma_start(rwp_sb, rwp[:, :, :])
            rb_sb = const.tile([P, E], f32)
            nc.sync.dma_start(rb_sb, rb[:, :])
            b1c_sb = const.tile([P, HKO], f32)
            nc.sync.dma_start(b1c_sb, b1c[:, :])
            b2c_sb = const.tile([P, DKO], f32)
            nc.sync.dma_start(b2c_sb, b2c[:, :])
            sb1c_sb = const.tile([P, SHKO], f32)
            nc.sync.dma_start(sb1c_sb, sb1c[:, :])
            sb2c_sb = const.tile([P, DKO], f32)
            nc.sync.dma_start(sb2c_sb, sb2c[:, :])
            oh_sb = const.tile([E, P], f32)
            nc.sync.dma_start(oh_sb, oh[:, :])

            w1_sb = wpool.tile([P, DKO, H], bf16)
            nc.sync.dma_start(w1_sb, w1p[:, :, :])
            sw1_sb = wpool.tile([P, DKO, SH], bf16)
            nc.sync.dma_start(sw1_sb, sw1p[:, :, :])
            sw2_sb = wpool.tile([P, SHKO, DIM], bf16)
            nc.sync.dma_start(sw2_sb, sw2p[:, :, :])
            ct_sb = wpool.tile([E, T], f32)

            with (
                tc.tile_pool(name="rx", bufs=2) as rxp,
                tc.tile_pool(name="rt", bufs=2) as rt,
                tc.tile_pool(name="rps", bufs=2, space="PSUM") as rps,
                tc.tile_pool(name="tps", bufs=2, space="PSUM") as tps,
            ):
                for tt in range(T // P):
                    rx = rxp.tile([P, DKO, P], f32, tag="rx")
                    nc.sync.dma_start(rx, xt32[:, :, tt * P : (tt + 1) * P])
                    pl = rps.tile([P, E], f32, tag="pl")
                    for ko in range(DKO):
                        nc.tensor.matmul(
                            pl,
                            rx[:, ko, :],
                            rwp_sb[:, ko, :],
                            start=(ko == 0),
                            stop=(ko == DKO - 1),
                        )
                    lg = rt.tile([P, E], f32, tag="lg")
                    nc.vector.tensor_add(lg, pl, rb_sb)
                    mx = rt.tile([P, 1], f32, tag="mx")
                    nc.vector.reduce_max(mx, lg, axis=AX.X)
                    nmx = rt.tile([P, 1], f32, tag="nmx")
                    nc.vector.tensor_scalar_mul(nmx, mx, -1.0)
                    ex = rt.tile([P, E], f32, tag="ex")
                    nc.scalar.activation(ex, lg, AF.Exp, bias=nmx)
                    sm = rt.tile([P, 1], f32, tag="sm")
                    nc.vector.reduce_sum(sm, ex, axis=AX.X)
                    rc = rt.tile([P, 1], f32, tag="rc")
                    nc.vector.reciprocal(rc, sm)
                    ge1 = rt.tile([P, E], f32, tag="ge1")
                    nc.vector.tensor_tensor(ge1, lg, mx.to_broadcast((P, E)), OP.is_ge)
                    big = rt.tile([P, E], f32, tag="big")
                    nc.vector.tensor_scalar_mul(big, ge1, 1e30)
                    lm = rt.tile([P, E], f32, tag="lm")
                    nc.vector.tensor_sub(lm, lg, big)
                    m2 = rt.tile([P, 1], f32, tag="m2")
                    nc.vector.reduce_max(m2, lm, axis=AX.X)
                    msk = rt.tile([P, E], f32, tag="msk")
                    nc.vector.tensor_tensor(msk, lg, m2.to_broadcast((P, E)), OP.is_ge)
                    pw = rt.tile([P, E], f32, tag="pw")
                    nc.vector.tensor_mul(pw, ex, msk)
                    nc.vector.tensor_tensor(pw, pw, rc.to_broadcast((P, E)), OP.mult)
                    cps = tps.tile([E, P], f32, tag="cps")
                    nc.tensor.transpose(cps, pw, ident)
                    nc.vector.tensor_copy(ct_sb[:, tt * P : (tt + 1) * P], cps)

            with (
                tc.tile_pool(name="xp", bufs=2) as xp,
                tc.tile_pool(name="w2p", bufs=2) as w2p,
                tc.tile_pool(name="hp", bufs=1) as hp,
                tc.tile_pool(name="cep", bufs=1) as cep,
                tc.tile_pool(name="op", bufs=1) as op_,
                tc.tile_pool(name="tp", bufs=1) as tp_,
                tc.tile_pool(name="dram", bufs=1, space="DRAM") as dram,
                tc.tile_pool(name="p1", bufs=2, space="PSUM") as p1p,
                tc.tile_pool(name="p2", bufs=2, space="PSUM") as p2p,
                tc.tile_pool(name="p2s", bufs=1, space="PSUM") as p2sp,
                tc.tile_pool(name="pc", bufs=1, space="PSUM") as pcp,
            ):
                for s in range(NSLICES):
                    t0 = s * SLICE
                    xt = xp.tile([P, DKO, SLICE], bf16, tag="xt")
                    nc.sync.dma_start(xt, xtbf[:, :, t0 : t0 + SLICE])
                    cei = pcp.tile([P, SLICE], f32, tag="cei")
                    nc.tensor.matmul(
                        cei, oh_sb, ct_sb[:, t0 : t0 + SLICE], start=True, stop=True
                    )
                    ce = cep.tile([P, SLICE], f32, tag="ce")
                    nc.vector.tensor_copy(ce, cei)

                    h = hp.tile([P, HKO + SHKO, SLICE], bf16, tag="h")
                    for hm in range(HKO):
                        ps = p1p.tile([P, SLICE], f32, tag="ps1")
                        for ko in range(DKO):
                            nc.tensor.matmul(
                                ps,
                                w1_sb[:, ko, hm * P : (hm + 1) * P],
                                xt[:, ko, :],
                                start=(ko == 0),
                                stop=(ko == DKO - 1),
                            )
                        nc.scalar.activation(
                            h[:, hm, :], ps, AF.Gelu, bias=b1c_sb[:, hm : hm + 1]
                        )
                    for sm_ in range(SHKO):
                        ps = p1p.tile([P, SLICE], f32, tag="ps1")
                        for ko in range(DKO):
                            nc.tensor.matmul(
                                ps,
                                sw1_sb[:, ko, sm_ * P : (sm_ + 1) * P],
                                xt[:, ko, :],
                                start=(ko == 0),
                                stop=(ko == DKO - 1),
                            )
                        nc.scalar.activation(
                            h[:, HKO + sm_, :], ps, AF.Gelu, bias=sb1c_sb[:, sm_ : sm_ + 1]
                        )

                    ob = op_.tile([P, DKO, SLICE], f32, tag="ob")
                    for dm in range(DKO):
                        w2t = w2p.tile([P, HKO, P], bf16, tag="w2t")
                        nc.sync.dma_start(w2t, w2b[dm, :, :, :])
                        ps2 = p2p.tile([P, SLICE], f32, tag="ps2")
                        for hk in range(HKO):
                            nc.tensor.matmul(
                                ps2,
                                w2t[:, hk, :],
                                h[:, hk, :],
                                start=(hk == 0),
                                stop=(hk == HKO - 1),
                            )
                        ps2s = p2sp.tile([P, SLICE], f32, tag="ps2s")
                        for sk in range(SHKO):
                            nc.tensor.matmul(
                                ps2s,
                                sw2_sb[:, sk, dm * P : (dm + 1) * P],
                                h[:, HKO + sk, :],
                                start=(sk == 0),
                                stop=(sk == SHKO - 1),
                            )
                        t1 = tp_.tile([P, SLICE], f32, tag="t1")
                        nc.scalar.activation(
                            t1, ps2, AF.Identity, bias=b2c_sb[:, dm : dm + 1]
                        )
                        nc.vector.tensor_mul(t1, t1, ce)
                        t2 = tp_.tile([P, SLICE], f32, tag="t2")
                        nc.scalar.activation(
                            t2, ps2s, AF.Identity, bias=sb2c_sb[:, dm : dm + 1]
                        )
                        nc.vector.tensor_add(ob[:, dm, :], t1, t2)

                    obd = dram.tile([DIM, SLICE], f32, tag=f"obd{s}", name=f"obd{s}")
                    nc.sync.dma_start(
                        obd.rearrange("(dm ki) t -> ki dm t", ki=P), ob
                    )
                    rso = dram.tile([P, SLICE], f32, tag=f"rso{s}", name=f"rso{s}")
                    if sim:
                        nc.sync.dma_start(rso[:, :], obd[0:P, :])
                    else:
                        nc.gpsimd.collective_compute(
                            "ReduceScatter",
                            OP.add,
                            replica_groups=[list(range(NCORES))],
                            ins=[obd.opt()],
                            outs=[rso.opt()],
                        )
                    nc.sync.dma_start(out[:, t0 : t0 + SLICE], rso[:, :])

    nc.finalize()
    return nc


def _get_nc():
    if "nc" not in _nc_cache:
        _nc_cache["nc"] = _build_nc()
    return _nc_cache["nc"]


def _prep_in_maps(x, router_w, router_b, w1, b1, w2, b2, sw1, sb1, sw2, sb2):
    xt = np.ascontiguousarray(x.reshape(T, DIM).astype(np.float32).T)
    xt32p = np.ascontiguousarray(xt.reshape(DKO, P, T).transpose(1, 0, 2))
    xtbfp = xt32p.astype(BF16)
    rwp = np.ascontiguousarray(
        router_w.astype(np.float32).reshape(DKO, P, E).transpose(1, 0, 2)
    )
    rb = np.tile(router_b.astype(np.float32)[None, :], (P, 1))
    rb = np.ascontiguousarray(rb)
    in_maps = []
    for e in range(NCORES):
        w1p = np.ascontiguousarray(
            w1[e].reshape(DKO, P, H).transpose(1, 0, 2)
        ).astype(BF16)
        b1ce = np.ascontiguousarray(b1[e].astype(np.float32).reshape(HKO, P).T)
        w2bb = np.ascontiguousarray(
            w2[e].reshape(HKO, P, DKO, P).transpose(2, 1, 0, 3)
        ).astype(BF16)
        b2ce = np.ascontiguousarray(b2[e].astype(np.float32).reshape(DKO, P).T)
        s0 = e * SH
        sw1pe = np.ascontiguousarray(
            sw1[:, s0 : s0 + SH].reshape(DKO, P, SH).transpose(1, 0, 2)
        ).astype(BF16)
        sb1ce = np.ascontiguousarray(
            sb1[s0 : s0 + SH].astype(np.float32).reshape(SHKO, P).T
        )
        sw2pe = np.ascontiguousarray(
            sw2[s0 : s0 + SH, :].reshape(SHKO, P, DIM).transpose(1, 0, 2)
        ).astype(BF16)
        sb2v = sb2 if e == 0 else np.zeros_like(sb2)
        sb2ce = np.ascontiguousarray(sb2v.astype(np.float32).reshape(DKO, P).T)
        ohm = np.zeros((E, P), np.float32)
        ohm[e, :] = 1.0
        in_maps.append(
            dict(
                xtbf=xtbfp,
                xt32=xt32p,
                rwp=rwp,
                rb=rb,
                w1p=w1p,
                b1c=b1ce,
                w2b=w2bb,
                b2c=b2ce,
                sw1p=sw1pe,
                sb1c=sb1ce,
                sw2p=sw2pe,
                sb2c=sb2ce,
                oh=ohm,
            )
        )
    return in_maps


C = 1152
CT = C // P
SLICES2 = [512, 512, 128]
BUFROWS = T + 8
CHUNK_BASE = [0, 1280, 2560]
CHUNK_TOK = [1280, 1280, 1536]
CHUNK_ROWS = [1288, 1288, 1544]
GROUP_CHUNKS = [[0], [0], [0, 1], [0, 1], [1, 2], [1, 2], [1, 2], [2], [2]]
GROUP_SLICE = [0, 0, 0, 0, 1, 1, 1, 1, 2]


def _build_phase1(sim=False):
    import concourse.mybir as mybir
    import concourse.tile as tile
    from concourse import bacc
    f32 = mybir.dt.float32
    bf16 = mybir.dt.bfloat16
    AF = mybir.ActivationFunctionType
    OP = mybir.AluOpType
    AX = mybir.AxisListType
    nc = bacc.Bacc("TRN2", target_bir_lowering=False, debug=False,
                   num_devices=1 if sim else NCORES)

    xt32 = nc.dram_tensor("xt32", [P, DKO, T], f32, kind="ExternalInput")
    rwp = nc.dram_tensor("rwp", [P, DKO, E], f32, kind="ExternalInput")
    rb = nc.dram_tensor("rb", [P, E], f32, kind="ExternalInput")
    sw1p = nc.dram_tensor("sw1p", [P, DKO, SH], bf16, kind="ExternalInput")
    sb1c = nc.dram_tensor("sb1c", [P, SHKO], f32, kind="ExternalInput")
    sw2p = nc.dram_tensor("sw2p", [P, SHKO, DIM], bf16, kind="ExternalInput")
    ct_out = nc.dram_tensor("ct", [T, E], f32, kind="ExternalOutput")
    shout = nc.dram_tensor("shout", [NSLICES, T // NSLICES // NCORES, DIM], bf16,
                           kind="ExternalOutput")

    with tile.TileContext(nc) as tc:
        with (
            tc.tile_pool(name="const", bufs=1) as const,
            tc.tile_pool(name="wpool", bufs=1) as wpool,
            tc.tile_pool(name="xp", bufs=3) as xp,
            tc.tile_pool(name="xbp", bufs=3) as xbp,
            tc.tile_pool(name="rt", bufs=3) as rt,
            tc.tile_pool(name="hp", bufs=3) as hp,
            tc.tile_pool(name="osb", bufs=4) as osb,
            tc.tile_pool(name="dram", bufs=1, space="DRAM") as dram,
            tc.tile_pool(name="rps", bufs=2, space="PSUM") as rps,
            tc.tile_pool(name="p1", bufs=2, space="PSUM") as p1p,
            tc.tile_pool(name="p2", bufs=2, space="PSUM") as p2p,
        ):
            rwp_sb = const.tile([P, DKO, E], f32)
            nc.sync.dma_start(rwp_sb, rwp[:, :, :])
            rb_sb = const.tile([P, E], f32)
            nc.sync.dma_start(rb_sb, rb[:, :])
            sb1c_sb = const.tile([P, SHKO], f32)
            nc.sync.dma_start(sb1c_sb, sb1c[:, :])
            sw1_sb = wpool.tile([P, DKO, SH], bf16)
            nc.sync.dma_start(sw1_sb, sw1p[:, :, :])
            sw2_sb = wpool.tile([P, SHKO, DIM], bf16)
            nc.sync.dma_start(sw2_sb, sw2p[:, :, :])

            CH = T // NSLICES // NCORES
            for s in range(NSLICES):
                t0 = s * SLICE
                xt = xp.tile([P, DKO, SLICE], f32, tag="xt")
                for ko2 in range(DKO):
                    nc.sync.dma_start(xt[:, ko2, :],
                                      xt32[:, ko2, t0 : t0 + SLICE])
                xtb = xbp.tile([P, DKO, SLICE], bf16, tag="xtb")
                nc.vector.tensor_copy(xtb, xt)

                NT4 = SLICE // P
                lg4 = rt.tile([P, NT4, E], f32, tag="lg4")
                for t4 in range(NT4):
                    pl = rps.tile([P, E], f32, tag="pl")
                    for ko in range(DKO):
                        nc.tensor.matmul(pl, xt[:, ko, t4 * P : (t4 + 1) * P],
                                         rwp_sb[:, ko, :],
                                         start=(ko == 0), stop=(ko == DKO - 1))
                    nc.vector.tensor_add(lg4[:, t4, :], pl, rb_sb)
                mx4 = rt.tile([P, NT4, 1], f32, tag="mx4")
                nc.vector.reduce_max(mx4, lg4, axis=AX.X)
                lgs = rt.tile([P, NT4, E], f32, tag="lgs")
                nc.vector.tensor_sub(lgs, lg4, mx4.to_broadcast((P, NT4, E)))
                ex4 = rt.tile([P, NT4, E], f32, tag="ex4")
                nc.scalar.activation(ex4, lgs, AF.Exp)
                sm4 = rt.tile([P, NT4, 1], f32, tag="sm4")
                nc.vector.reduce_sum(sm4, ex4, axis=AX.X)
                rc4 = rt.tile([P, NT4, 1], f32, tag="rc4")
                nc.vector.reciprocal(rc4, sm4)
                ge1 = rt.tile([P, NT4, E], f32, tag="ge1")
                nc.vector.tensor_scalar(ge1, lgs, 0.0, 1e30,
                                        OP.is_ge, OP.mult)
                lm4 = rt.tile([P, NT4, E], f32, tag="lm4")
                nc.vector.tensor_sub(lm4, lgs, ge1)
                m24 = rt.tile([P, NT4, 1], f32, tag="m24")
                nc.vector.reduce_max(m24, lm4, axis=AX.X)
                msk4 = rt.tile([P, NT4, E], f32, tag="msk4")
                nc.vector.tensor_tensor(msk4, lgs, m24.to_broadcast((P, NT4, E)),
                                        OP.is_ge)
                pw4 = rt.tile([P, NT4, E], f32, tag="pw4")
                nc.vector.tensor_mul(pw4, ex4, msk4)
                nc.vector.tensor_tensor(pw4, pw4, rc4.to_broadcast((P, NT4, E)),
                                        OP.mult)
                nc.sync.dma_start(
                    ct_out[s * SLICE : (s + 1) * SLICE, :].rearrange(
                        "(t4 p) e -> p t4 e", p=P),
                    pw4)

                hs = hp.tile([P, SHKO, SLICE], bf16, tag="hs")
                for sm_ in range(SHKO):
                    ps = p1p.tile([P, SLICE], f32, tag="ps1")
                    for ko in range(DKO):
                        nc.tensor.matmul(ps, sw1_sb[:, ko, sm_ * P : (sm_ + 1) * P],
                                         xtb[:, ko, :],
                                         start=(ko == 0), stop=(ko == DKO - 1))
                    nc.scalar.activation(hs[:, sm_, :], ps, AF.Gelu,
                                         bias=sb1c_sb[:, sm_ : sm_ + 1])
                shb = dram.tile([SLICE, DIM], bf16, tag=f"shb{s}", name=f"shb{s}")
                for tt in range(SLICE // P):
                    o_sb = osb.tile([P, DIM], bf16, tag="o_sb")
                    ps2a = p2p.tile([P, 512], f32, tag="ps2_0")
                    ps2b = p2p.tile([P, 512], f32, tag="ps2_1")
                    for sk in range(SHKO):
                        nc.tensor.matmul(ps2a, hs[:, sk, tt * P : (tt + 1) * P],
                                         sw2_sb[:, sk, 0:512],
                                         start=(sk == 0), stop=(sk == SHKO - 1))
                        nc.tensor.matmul(ps2b, hs[:, sk, tt * P : (tt + 1) * P],
                                         sw2_sb[:, sk, 512:1024],
                                         start=(sk == 0), stop=(sk == SHKO - 1))
                    nc.vector.tensor_copy(o_sb[:, 0:512], ps2a)
                    nc.vector.tensor_copy(o_sb[:, 512:1024], ps2b)
                    nc.sync.dma_start(shb[tt * P : (tt + 1) * P, :], o_sb)
                shrs = dram.tile([CH, DIM], bf16, tag=f"shrs{s}", name=f"shrs{s}")
                if sim:
                    nc.sync.dma_start(shrs[:, :], shb[0:CH, :])
                else:
                    nc.gpsimd.collective_compute(
                        "ReduceScatter", OP.add,
                        replica_groups=[list(range(NCORES))],
                        ins=[shb.opt()], outs=[shrs.opt()])
                nc.sync.dma_start(shout[s, :, :], shrs[:, :])

    nc.finalize()
    return nc


def _build_phase2(sim=False):
    import concourse.mybir as mybir
    import concourse.tile as tile
    from concourse import bacc
    from concourse.bass import IndirectOffsetOnAxis
    f32 = mybir.dt.float32
    bf16 = mybir.dt.bfloat16
    i32 = mybir.dt.int32
    AF = mybir.ActivationFunctionType
    OP = mybir.AluOpType
    nc = bacc.Bacc("TRN2", target_bir_lowering=False, debug=False,
                   num_devices=1 if sim else NCORES)

    xg = nc.dram_tensor("xg", [P, DKO, C], bf16, kind="ExternalInput")
    w1p = nc.dram_tensor("w1p", [HKO, P, DKO, P], bf16, kind="ExternalInput")
    b1c = nc.dram_tensor("b1c", [P, HKO], f32, kind="ExternalInput")
    w2p = nc.dram_tensor("w2p", [P, HKO, DIM], bf16, kind="ExternalInput")
    b2r = nc.dram_tensor("b2r", [1, DIM], f32, kind="ExternalInput")
    ceg = nc.dram_tensor("ceg", [P, CT], f32, kind="ExternalInput")
    idxs = [nc.dram_tensor(f"idx{c}", [P, CT], i32, kind="ExternalInput")
            for c in range(3)]
    EOUT = sum(r // NCORES for r in CHUNK_ROWS)
    eout = nc.dram_tensor("eout", [EOUT, DIM], bf16, kind="ExternalOutput")

    with tile.TileContext(nc) as tc:
        with (
            tc.tile_pool(name="const", bufs=1) as const,
            tc.tile_pool(name="wpool", bufs=1) as wpool,
            tc.tile_pool(name="hp", bufs=1) as hp,
            tc.tile_pool(name="ysb", bufs=2) as ysb,
            tc.tile_pool(name="dram", bufs=1, space="DRAM") as dram,
            tc.tile_pool(name="p1", bufs=3, space="PSUM") as p1p,
            tc.tile_pool(name="p2", bufs=2, space="PSUM") as p2p,
        ):
            b1c_sb = const.tile([P, HKO], f32)
            nc.sync.dma_start(b1c_sb, b1c[:, :])
            b2r_sb = const.tile([1, DIM], f32)
            nc.sync.dma_start(b2r_sb, b2r[:, :])
            ceg_sb = const.tile([P, CT], f32)
            nc.sync.dma_start(ceg_sb, ceg[:, :])
            idx_sbs = []
            for c in range(3):
                idx_sb_c = const.tile([P, CT], i32, name=f"idx_sb{c}")
                nc.sync.dma_start(idx_sb_c, idxs[c][:, :])
                idx_sbs.append(idx_sb_c)
            ones1 = const.tile([1, P], f32)
            nc.vector.memset(ones1, 1.0)
            zero_sb = const.tile([P, DIM], bf16)
            nc.vector.memset(zero_sb, 0.0)

            xg_sb = wpool.tile([P, DKO, C], bf16)
            c0 = 0
            for Wx in SLICES2:
                nc.sync.dma_start(xg_sb[:, :, c0 : c0 + Wx], xg[:, :, c0 : c0 + Wx])
                c0 += Wx
            w1_sb = wpool.tile([P, DKO, H], bf16)
            for hm in range(HKO):
                nc.sync.dma_start(w1_sb[:, :, hm * P : (hm + 1) * P],
                                  w1p[hm, :, :, :])
            w2_sb = wpool.tile([P, HKO, DIM], bf16)
            nc.sync.dma_start(w2_sb, w2p[:, :, :])

            bufs = []
            for c in range(3):
                buf_c = dram.tile([CHUNK_ROWS[c], DIM], bf16, name=f"buf{c}",
                                  tag=f"buf{c}")
                for r0 in range(0, CHUNK_TOK[c], P):
                    nc.sync.dma_start(buf_c[r0 : r0 + P, :], zero_sb)
                bufs.append(buf_c)

            sl0 = 0
            for s, W in enumerate(SLICES2):
                h = hp.tile([P, HKO, 512], bf16, tag="h")
                for hm in range(HKO):
                    ps = p1p.tile([P, 512], f32, tag="ps1")
                    for ko in range(DKO):
                        nc.tensor.matmul(ps[:, :W], w1_sb[:, ko, hm * P : (hm + 1) * P],
                                         xg_sb[:, ko, sl0 : sl0 + W],
                                         start=(ko == 0), stop=(ko == DKO - 1))
                    nc.scalar.activation(h[:, hm, :W], ps[:, :W], AF.Gelu,
                                         bias=b1c_sb[:, hm : hm + 1])
                for tt in range(W // P):
                    gtt = sl0 // P + tt
                    y_sb = ysb.tile([P, DIM], bf16, tag="y_sb")
                    ps2a = p2p.tile([P, 512], f32, tag="ps2_0")
                    ps2b = p2p.tile([P, 512], f32, tag="ps2_1")
                    for hk in range(HKO):
                        nc.tensor.matmul(ps2a, h[:, hk, tt * P : (tt + 1) * P],
                                         w2_sb[:, hk, 0:512],
                                         start=(hk == 0), stop=False)
                        nc.tensor.matmul(ps2b, h[:, hk, tt * P : (tt + 1) * P],
                                         w2_sb[:, hk, 512:1024],
                                         start=(hk == 0), stop=False)
                    nc.tensor.matmul(ps2a, ones1[0:1, 0:P], b2r_sb[0:1, 0:512],
                                     start=False, stop=True)
                    nc.tensor.matmul(ps2b, ones1[0:1, 0:P], b2r_sb[0:1, 512:1024],
                                     start=False, stop=True)
                    nc.vector.tensor_tensor(
                        y_sb[:, 0:512], ps2a,
                        ceg_sb[:, gtt : gtt + 1].to_broadcast((P, 512)), OP.mult)
                    nc.vector.tensor_tensor(
                        y_sb[:, 512:1024], ps2b,
                        ceg_sb[:, gtt : gtt + 1].to_broadcast((P, 512)), OP.mult)
                    for c in GROUP_CHUNKS[gtt]:
                        nc.gpsimd.indirect_dma_start(
                            out=bufs[c][:, :],
                            out_offset=IndirectOffsetOnAxis(
                                ap=idx_sbs[c][:, gtt : gtt + 1], axis=0),
                            in_=y_sb[:, :],
                            in_offset=None)
                sl0 += W
                csz = CHUNK_ROWS[s] // NCORES
                rs_c = dram.tile([csz, DIM], bf16, name=f"rs{s}", tag=f"rs{s}")
                if sim:
                    nc.sync.dma_start(rs_c[:, :], bufs[s][0:csz, :])
                else:
                    nc.gpsimd.collective_compute(
                        "ReduceScatter", OP.add,
                        replica_groups=[list(range(NCORES))],
                        ins=[bufs[s].opt()], outs=[rs_c.opt()])
                e0 = sum(r // NCORES for r in CHUNK_ROWS[:s])
                nc.sync.dma_start(eout[e0 : e0 + csz, :], rs_c[:, :])

    nc.finalize()
    return nc


def _get(name, builder):
    if name not in _nc_cache:
        _nc_cache[name] = builder()
    return _nc_cache[name]


def _prep_phase1(x, router_w, router_b, sw1, sb1, sw2, sb2):
    xt = np.ascontiguousarray(x.reshape(T, DIM).astype(np.float32).T)
    xt32p = np.ascontiguousarray(xt.reshape(DKO, P, T).transpose(1, 0, 2))
    rwp = np.ascontiguousarray(router_w.astype(np.float32).reshape(DKO, P, E).transpose(1, 0, 2))
    rb = np.ascontiguousarray(np.tile(router_b.astype(np.float32)[None, :], (P, 1)))
    maps = []
    for e in range(NCORES):
        s0 = e * SH
        sw1pe = np.ascontiguousarray(sw1[:, s0:s0 + SH].reshape(DKO, P, SH).transpose(1, 0, 2)).astype(BF16)
        sb1ce = np.ascontiguousarray(sb1[s0:s0 + SH].astype(np.float32).reshape(SHKO, P).T)
        sw2pe = np.ascontiguousarray(sw2[s0:s0 + SH, :].reshape(SHKO, P, DIM).transpose(1, 0, 2)).astype(BF16)
        maps.append(dict(xt32=xt32p, rwp=rwp, rb=rb, sw1p=sw1pe,
                         sb1c=sb1ce, sw2p=sw2pe))
    return maps, xt


def _prep_phase2(ct, xt, w1, b1, w2, b2):
    maps = []
    for e in range(NCORES):
        sel = np.nonzero(ct[:, e])[0].astype(np.int64)
        if len(sel) > C:
            return None
        npad = C - len(sel)
        selp = np.concatenate([sel, np.zeros(npad, np.int64)])
        nreal = len(sel)
        cev = np.concatenate([ct[sel, e].astype(np.float32), np.zeros(npad, np.float32)])
        slot_chunk = np.digitize(sel, CHUNK_BASE[1:])
        for g in range(CT):
            lo, hi = g * P, min((g + 1) * P, nreal)
            if lo >= hi:
                continue
            if not set(np.unique(slot_chunk[lo:hi])) <= set(GROUP_CHUNKS[g]):
                return None
        idxps = []
        for c in range(3):
            dump = CHUNK_TOK[c] + (np.arange(C) % 8)
            inchunk = np.zeros(C, bool)
            inchunk[:nreal] = slot_chunk == c
            dest = np.where(inchunk, selp - CHUNK_BASE[c], dump)
            idxps.append(np.ascontiguousarray(dest.reshape(CT, P).T.astype(np.int32)))
        xge = xt[:, selp]
        xgp = np.ascontiguousarray(xge.reshape(DKO, P, C).transpose(1, 0, 2)).astype(BF16)
        w1pe = np.ascontiguousarray(
            w1[e].reshape(DKO, P, HKO, P).transpose(2, 1, 0, 3)).astype(BF16)
        b1ce = np.ascontiguousarray(b1[e].astype(np.float32).reshape(HKO, P).T)
        w2pe = np.ascontiguousarray(w2[e].reshape(HKO, P, DIM).transpose(1, 0, 2)).astype(BF16)
        cegp = np.ascontiguousarray(cev.reshape(CT, P).T)
        maps.append(dict(xg=xgp, w1p=w1pe, b1c=b1ce, w2p=w2pe,
                         b2r=np.ascontiguousarray(b2[e].astype(np.float32)[None, :]),
                         ceg=cegp, idx0=idxps[0], idx1=idxps[1], idx2=idxps[2]))
    return maps


def _run_spmd(nc, in_maps, trace=False):
    from concourse.bass_utils import run_bass_kernel_spmd
    return run_bass_kernel_spmd(nc, in_maps, core_ids=list(range(NCORES)), trace=trace)


def _kernel_dense(x, router_w, router_b, w1, b1, w2, b2, sw1, sb1, sw2, sb2):
    in_maps = _prep_in_maps(x, router_w, router_b, w1, b1, w2, b2, sw1, sb1, sw2, sb2)
    res = _run_spmd(_get("dense", _build_nc), in_maps)
    outT = np.concatenate([r["out"] for r in res.results], axis=0)
    return np.ascontiguousarray(outT.T).reshape(2, 2048, DIM)


def _kernel_sparse(x, router_w, router_b, w1, b1, w2, b2, sw1, sb1, sw2, sb2):
    maps1, xt = _prep_phase1(x, router_w, router_b, sw1, sb1, sw2, sb2)
    res1 = _run_spmd(_get("p1", _build_phase1), maps1)
    ct = res1.results[0]["ct"]

    CH = T // NSLICES // NCORES
    shared = np.zeros((T, DIM), np.float32)
    for j in range(NCORES):
        sh = res1.results[j]["shout"].astype(np.float32)
        for s in range(NSLICES):
            r0 = s * SLICE + j * CH
            shared[r0:r0 + CH] = sh[s]
    shared += sb2.astype(np.float32)[None, :]
    maps2 = _prep_phase2(ct, xt, w1, b1, w2, b2)
    if maps2 is None:
        return _kernel_dense(x, router_w, router_b, w1, b1, w2, b2,
                             sw1, sb1, sw2, sb2)
    res2 = _run_spmd(_get("p2", _build_phase2), maps2)
    expert = np.empty((T, DIM), np.float32)
    e0 = 0
    for c in range(3):
        csz = CHUNK_ROWS[c] // NCORES
        seg = np.concatenate(
            [r["eout"][e0 : e0 + csz] for r in res2.results], axis=0)
        expert[CHUNK_BASE[c] : CHUNK_BASE[c] + CHUNK_TOK[c]] = seg[: CHUNK_TOK[c]]
        e0 += csz
    return (expert + shared).reshape(2, 2048, DIM)


def kernel(x, router_w, router_b, w1, b1, w2, b2, sw1, sb1, sw2, sb2):
    import os
    if os.environ.get("MOE_DENSE"):
        return _kernel_dense(x, router_w, router_b, w1, b1, w2, b2, sw1, sb1, sw2, sb2)
    return _kernel_sparse(x, router_w, router_b, w1, b1, w2, b2, sw1, sb1, sw2, sb2)



# revision 11
# speedup vs baseline: 1.3750x; 1.3750x over previous
import sys

if "/opt/trn_rl_repo" not in sys.path:
    sys.path.insert(0, "/opt/trn_rl_repo")

import numpy as np
import ml_dtypes

DIM = 1024
E = 8
H = 4096
T = 4096
NCORES = 8
P = 128
DKO = DIM // P
HKO = H // P
SH = H // NCORES
SHKO = SH // P
SLICE = 512
NSLICES = T // SLICE

C = 1091
CSL = [512, 512, C - 1024]
CS0 = [0, 512, 1024]
BLK = T // NCORES

BF16 = ml_dtypes.bfloat16

_nc_cache = {}


def _build_nc(sim=False):
    import concourse.mybir as mybir
    import concourse.tile as tile
    from concourse import bacc
    from concourse.masks import make_identity

    f32 = mybir.dt.float32
    bf16 = mybir.dt.bfloat16
    AF = mybir.ActivationFunctionType
    OP = mybir.AluOpType
    AX = mybir.AxisListType

    ndev = 1 if sim else NCORES
    nc = bacc.Bacc("TRN2", target_bir_lowering=False, debug=False, num_devices=ndev)

    xtbf = nc.dram_tensor("xtbf", [P, DKO, T], bf16, kind="ExternalInput")
    xt32 = nc.dram_tensor("xt32", [P, DKO, T], f32, kind="ExternalInput")
    rwp = nc.dram_tensor("rwp", [P, DKO, E], f32, kind="ExternalInput")
    rb = nc.dram_tensor("rb", [P, E], f32, kind="ExternalInput")
    w1p = nc.dram_tensor("w1p", [P, DKO, H], bf16, kind="ExternalInput")
    b1c = nc.dram_tensor("b1c", [P, HKO], f32, kind="ExternalInput")
    w2b = nc.dram_tensor("w2b", [DKO, P, HKO, P], bf16, kind="ExternalInput")
    b2c = nc.dram_tensor("b2c", [P, DKO], f32, kind="ExternalInput")
    sw1p = nc.dram_tensor("sw1p", [P, DKO, SH], bf16, kind="ExternalInput")
    sb1c = nc.dram_tensor("sb1c", [P, SHKO], f32, kind="ExternalInput")
    sw2p = nc.dram_tensor("sw2p", [P, SHKO, DIM], bf16, kind="ExternalInput")
    sb2c = nc.dram_tensor("sb2c", [P, DKO], f32, kind="ExternalInput")
    oh = nc.dram_tensor("oh", [E, P], f32, kind="ExternalInput")
    out = nc.dram_tensor("out", [P, T], f32, kind="ExternalOutput")

    with tile.TileContext(nc) as tc:
        with (
            tc.tile_pool(name="const", bufs=1) as const,
            tc.tile_pool(name="wpool", bufs=1) as wpool,
        ):
            ident = const.tile([P, P], f32)
            make_identity(nc, ident)
            rwp_sb = const.tile([P, DKO, E], f32)
            nc.sync.dma_start(rwp_sb, rwp[:, :, :])
            rb_sb = const.tile([P, E], f32)
            nc.sync.dma_start(rb_sb, rb[:, :])
            b1c_sb = const.tile([P, HKO], f32)
            nc.sync.dma_start(b1c_sb, b1c[:, :])
            b2c_sb = const.tile([P, DKO], f32)
            nc.sync.dma_start(b2c_sb, b2c[:, :])
            sb1c_sb = const.tile([P, SHKO], f32)
            nc.sync.dma_start(sb1c_sb, sb1c[:, :])
            sb2c_sb = const.tile([P, DKO], f32)
            nc.sync.dma_start(sb2c_sb, sb2c[:, :])
            oh_sb = const.tile([E, P], f32)
            nc.sync.dma_start(oh_sb, oh[:, :])

            w1_sb = wpool.tile([P, DKO, H], bf16)
            nc.sync.dma_start(w1_sb, w1p[:, :, :])
            sw1_sb = wpool.tile([P, DKO, SH], bf16)
            nc.sync.dma_start(sw1_sb, sw1p[:, :, :])
            sw2_sb = wpool.tile([P, SHKO, DIM], bf16)
            nc.sync.dma_start(sw2_sb, sw2p[:, :, :])
            ct_sb = wpool.tile([E, T], f32)

            with (
                tc.tile_pool(name="rx", bufs=2) as rxp,
                tc.tile_pool(name="rt", bufs=2) as rt,
                tc.tile_pool(name="rps", bufs=2, space="PSUM") as rps,
                tc.tile_pool(name="tps", bufs=2, space="PSUM") as tps,
            ):
                for tt in range(T // P):
                    rx = rxp.tile([P, DKO, P], f32, tag="rx")
                    nc.sync.dma_start(rx, xt32[:, :, tt * P : (tt + 1) * P])
                    pl = rps.tile([P, E], f32, tag="pl")
                    for ko in range(DKO):
                        nc.tensor.matmul(
                            pl,
                            rx[:, ko, :],
                            rwp_sb[:, ko, :],
                            start=(ko == 0),
                            stop=(ko == DKO - 1),
                        )
                    lg = rt.tile([P, E], f32, tag="lg")
                    nc.vector.tensor_add(lg, pl, rb_sb)
                    mx = rt.tile([P, 1], f32, tag="mx")
                    nc.vector.reduce_max(mx, lg, axis=AX.X)
                    nmx = rt.tile([P, 1], f32, tag="nmx")
                    nc.vector.tensor_scalar_mul(nmx, mx, -1.0)
                    ex = rt.tile([P, E], f32, tag="ex")
                    nc.scalar.activation(ex, lg, AF.Exp, bias=nmx)
                    sm = rt.tile([P, 1], f32, tag="sm")
                    nc.vector.reduce_sum(sm, ex, axis=AX.X)
                    rc = rt.tile([P, 1], f32, tag="rc")
                    nc.vector.reciprocal(rc, sm)
                    ge1 = rt.tile([P, E], f32, tag="ge1")
                    nc.vector.tensor_tensor(ge1, lg, mx.to_broadcast((P, E)), OP.is_ge)
                    big = rt.tile([P, E], f32, tag="big")
                    nc.vector.tensor_scalar_mul(big, ge1, 1e30)
                    lm = rt.tile([P, E], f32, tag="lm")
                    nc.vector.tensor_sub(lm, lg, big)
                    m2 = rt.tile([P, 1], f32, tag="m2")
                    nc.vector.reduce_max(m2, lm, axis=AX.X)
                    msk = rt.tile([P, E], f32, tag="msk")
                    nc.vector.tensor_tensor(msk, lg, m2.to_broadcast((P, E)), OP.is_ge)
                    pw = rt.tile([P, E], f32, tag="pw")
                    nc.vector.tensor_mul(pw, ex, msk)
                    nc.vector.tensor_tensor(pw, pw, rc.to_broadcast((P, E)), OP.mult)
                    cps = tps.tile([E, P], f32, tag="cps")
                    nc.tensor.transpose(cps, pw, ident)
                    nc.vector.tensor_copy(ct_sb[:, tt * P : (tt + 1) * P], cps)

            with (
                tc.tile_pool(name="xp", bufs=2) as xp,
                tc.tile_pool(name="w2p", bufs=2) as w2p,
                tc.tile_pool(name="hp", bufs=1) as hp,
                tc.tile_pool(name="cep", bufs=1) as cep,
                tc.tile_pool(name="op", bufs=1) as op_,
                tc.tile_pool(name="tp", bufs=1) as tp_,
                tc.tile_pool(name="dram", bufs=1, space="DRAM") as dram,
                tc.tile_pool(name="p1", bufs=2, space="PSUM") as p1p,
                tc.tile_pool(name="p2", bufs=2, space="PSUM") as p2p,
                tc.tile_pool(name="p2s", bufs=1, space="PSUM") as p2sp,
                tc.tile_pool(name="pc", bufs=1, space="PSUM") as pcp,
            ):
                for s in range(NSLICES):
                    t0 = s * SLICE
                    xt = xp.tile([P, DKO, SLICE], bf16, tag="xt")
                    nc.sync.dma_start(xt, xtbf[:, :, t0 : t0 + SLICE])
                    cei = pcp.tile([P, SLICE], f32, tag="cei")
                    nc.tensor.matmul(
                        cei, oh_sb, ct_sb[:, t0 : t0 + SLICE], start=True, stop=True
                    )
                    ce = cep.tile([P, SLICE], f32, tag="ce")
                    nc.vector.tensor_copy(ce, cei)

                    h = hp.tile([P, HKO + SHKO, SLICE], bf16, tag="h")
                    for hm in range(HKO):
                        ps = p1p.tile([P, SLICE], f32, tag="ps1")
                        for ko in range(DKO):
                            nc.tensor.matmul(
                                ps,
                                w1_sb[:, ko, hm * P : (hm + 1) * P],
                                xt[:, ko, :],
                                start=(ko == 0),
                                stop=(ko == DKO - 1),
                            )
                        nc.scalar.activation(
                            h[:, hm, :], ps, AF.Gelu, bias=b1c_sb[:, hm : hm + 1]
                        )
                    for sm_ in range(SHKO):
                        ps = p1p.tile([P, SLICE], f32, tag="ps1")
                        for ko in range(DKO):
                            nc.tensor.matmul(
                                ps,
                                sw1_sb[:, ko, sm_ * P : (sm_ + 1) * P],
                                xt[:, ko, :],
                                start=(ko == 0),
                                stop=(ko == DKO - 1),
                            )
                        nc.scalar.activation(
                            h[:, HKO + sm_, :], ps, AF.Gelu, bias=sb1c_sb[:, sm_ : sm_ + 1]
                        )

                    ob = op_.tile([P, DKO, SLICE], f32, tag="ob")
                    for dm in range(DKO):
                        w2t = w2p.tile([P, HKO, P], bf16, tag="w2t")
                        nc.sync.dma_start(w2t, w2b[dm, :, :, :])
                        ps2 = p2p.tile([P, SLICE], f32, tag="ps2")
                        for hk in range(HKO):
                            nc.tensor.matmul(
                                ps2,
                                w2t[:, hk, :],
                                h[:, hk, :],
                                start=(hk == 0),
                                stop=(hk == HKO - 1),
                            )
                        ps2s = p2sp.tile([P, SLICE], f32, tag="ps2s")
                        for sk in range(SHKO):
                            nc.tensor.matmul(
                                ps2s,
                                sw2_sb[:, sk, dm * P : (dm + 1) * P],
                                h[:, HKO + sk, :],
                                start=(sk == 0),
                                stop=(sk == SHKO - 1),
                            )
                        t1 = tp_.tile([P, SLICE], f32, tag="t1")
                        nc.scalar.activation(
                            t1, ps2, AF.Identity, bias=b2c_sb[:, dm : dm + 1]
                        )
                        nc.vector.tensor_mul(t1, t1, ce)
                        t2 = tp_.tile([P, SLICE], f32, tag="t2")
                        nc.scalar.activation(
                            t2, ps2s, AF.Identity, bias=sb2c_sb[:, dm : dm + 1]
                        )
                        nc.vector.tensor_add(ob[:, dm, :], t1, t2)

                    obd = dram.tile([DIM, SLICE], f32, tag=f"obd{s}", name=f"obd{s}")
                    nc.sync.dma_start(
                        obd.rearrange("(dm ki) t -> ki dm t", ki=P), ob
                    )
                    rso = dram.tile([P, SLICE], f32, tag=f"rso{s}", name=f"rso{s}")
                    if sim:
                        nc.sync.dma_start(rso[:, :], obd[0:P, :])
                    else:
                        nc.gpsimd.collective_compute(
                            "ReduceScatter",
                            OP.add,
                            replica_groups=[list(range(NCORES))],
                            ins=[obd.opt()],
                            outs=[rso.opt()],
                        )
                    nc.sync.dma_start(out[:, t0 : t0 + SLICE], rso[:, :])

    nc.finalize()
    return nc


def _build_moe(sim=False):
    import concourse.mybir as mybir
    import concourse.tile as tile
    from concourse import bacc

    f32 = mybir.dt.float32
    bf16 = mybir.dt.bfloat16
    AF = mybir.ActivationFunctionType
    OP = mybir.AluOpType

    nc = bacc.Bacc("TRN2", target_bir_lowering=False, debug=False,
                   num_devices=1 if sim else NCORES)

    xg0 = nc.dram_tensor("xg0", [P, DKO, CSL[0]], bf16, kind="ExternalInput")
    xg1 = nc.dram_tensor("xg1", [P, DKO, CSL[1]], bf16, kind="ExternalInput")
    xg2 = nc.dram_tensor("xg2", [P, DKO, CSL[2]], bf16, kind="ExternalInput")
    xb = nc.dram_tensor("xb", [P, DKO, BLK], bf16, kind="ExternalInput")
    ceb = nc.dram_tensor("ceb", [P, C], f32, kind="ExternalInput")
    w1s = nc.dram_tensor("w1s", [HKO, P, DKO, P], bf16, kind="ExternalInput")
    b1c = nc.dram_tensor("b1c", [P, HKO], f32, kind="ExternalInput")
    w2s = nc.dram_tensor("w2s", [DKO, P, HKO, P], bf16, kind="ExternalInput")
    sw1s = nc.dram_tensor("sw1s", [HKO, P, DKO, P], bf16, kind="ExternalInput")
    sb1c = nc.dram_tensor("sb1c", [P, HKO], f32, kind="ExternalInput")
    sw2p = nc.dram_tensor("sw2p", [P, HKO, DIM], bf16, kind="ExternalInput")
    ye = nc.dram_tensor("ye", [DKO, P, C], bf16, kind="ExternalOutput")
    ys = nc.dram_tensor("ys", [BLK, DIM], bf16, kind="ExternalOutput")

    with tile.TileContext(nc) as tc:
        with (
            tc.tile_pool(name="const", bufs=1) as const,
            tc.tile_pool(name="xgp", bufs=1) as xgp,
            tc.tile_pool(name="hbuf", bufs=1) as hbuf,
            tc.tile_pool(name="w1p", bufs=4) as w1p,
            tc.tile_pool(name="w2p", bufs=2) as w2p,
            tc.tile_pool(name="sw1p", bufs=4) as sw1p,
            tc.tile_pool(name="sw2pool", bufs=4) as sw2pool,
            tc.tile_pool(name="yop", bufs=2) as yop,
            tc.tile_pool(name="ysop", bufs=2) as ysop,
            tc.tile_pool(name="warm", bufs=1) as warm,
            tc.tile_pool(name="ph", bufs=2, space="PSUM") as php,
        ):
            wza = warm.tile([P, P], bf16)
            nc.vector.memset(wza, 0.0)
            wzb = warm.tile([P, 512], bf16)
            nc.vector.memset(wzb, 0.0)

            swblk0 = sw1p.tile([P, DKO, P], bf16, tag="swblk", name="swblk0")
            nc.sync.dma_start(swblk0, sw1s[0, :, :, :])
            xb_sb = xgp.tile([P, DKO, BLK], bf16, name="xb_sb")
            nc.sync.dma_start(xb_sb[:, 0 : DKO // 2, :], xb[:, 0 : DKO // 2, :])
            sb1c_sb = const.tile([P, HKO], f32)
            nc.sync.dma_start(sb1c_sb, sb1c[:, :])
            nc.sync.dma_start(xb_sb[:, DKO // 2 : DKO, :], xb[:, DKO // 2 : DKO, :])

            h = hbuf.tile([P, HKO, C], bf16)
            hs = hbuf.tile([P, HKO, BLK], bf16)
            sw2b_sb = hbuf.tile([P, HKO, 512], bf16)
            xgs = [xgp.tile([P, DKO, CSL[i]], bf16, name=f"xg_sb{i}")
                   for i in range(3)]
            b1c_sb = const.tile([P, HKO], f32)
            ce_sb = const.tile([P, C], f32)

            wps = php.tile([P, 512], f32, tag="ph3", name="wps")
            for k in range(10):
                nc.tensor.matmul(wps, wza, wzb,
                                 start=(k == 0), stop=(k == 9))

            for hm in range(HKO):
                if hm == 0:
                    swblk = swblk0
                else:
                    swblk = sw1p.tile([P, DKO, P], bf16, tag="swblk")
                    nc.sync.dma_start(swblk, sw1s[hm, :, :, :])
                ps = php.tile([P, BLK], f32, tag="ph0", name="psh")
                for ko in range(DKO):
                    nc.tensor.matmul(ps, swblk[:, ko, :], xb_sb[:, ko, :],
                                     start=(ko == 0), stop=(ko == DKO - 1))
                nc.scalar.activation(hs[:, hm, :], ps, AF.Gelu,
                                     bias=sb1c_sb[:, hm : hm + 1])
                if hm == 6:
                    nc.sync.dma_start(xgs[0], xg0[:, :, :])
                elif hm == 11:
                    nc.sync.dma_start(xgs[1], xg1[:, :, :])
                elif hm == 16:
                    nc.sync.dma_start(xgs[2], xg2[:, :, :])
                elif hm == 21:
                    nc.sync.dma_start(b1c_sb, b1c[:, :])
                    nc.sync.dma_start(ce_sb, ceb[:, :])

            for hm in range(HKO):
                w1blk = w1p.tile([P, DKO, P], bf16, tag="w1blk")
                nc.sync.dma_start(w1blk, w1s[hm, :, :, :])
                pss = [php.tile([P, 512], f32, tag=f"ph{s}", name=f"ph{s}")
                       for s in range(3)]
                for ko in range(DKO):
                    for s in range(3):
                        nc.tensor.matmul(pss[s][:, : CSL[s]],
                                         w1blk[:, ko, :],
                                         xgs[s][:, ko, :],
                                         start=(ko == 0), stop=(ko == DKO - 1))
                for s in range(3):
                    nc.scalar.activation(h[:, hm, CS0[s] : CS0[s] + CSL[s]],
                                         pss[s][:, : CSL[s]], AF.Gelu,
                                         bias=b1c_sb[:, hm : hm + 1])
                if hm == 24:
                    w2pre = w2p.tile([P, HKO, P], bf16, tag="w2blk",
                                     name="w2pre")
                    nc.sync.dma_start(w2pre, w2s[0, :, :, :])

            for dm in range(DKO):
                if dm == 0:
                    w2blk = w2pre
                else:
                    w2blk = w2p.tile([P, HKO, P], bf16, tag="w2blk")
                    nc.sync.dma_start(w2blk, w2s[dm, :, :, :])
                pss = [php.tile([P, 512], f32, tag=f"ph{s}", name=f"ph{s}")
                       for s in range(3)]
                for hk in range(HKO):
                    for s in range(3):
                        nc.tensor.matmul(pss[s][:, : CSL[s]],
                                         w2blk[:, hk, :],
                                         h[:, hk, CS0[s] : CS0[s] + CSL[s]],
                                         start=(hk == 0), stop=(hk == HKO - 1))
                for s in range(3):
                    yo = yop.tile([P, CSL[s]], bf16, tag=f"yo{s}")
                    nc.vector.tensor_tensor(yo, pss[s][:, : CSL[s]],
                                            ce_sb[:, CS0[s] : CS0[s] + CSL[s]],
                                            OP.mult)
                    nc.sync.dma_start(ye[dm, :, CS0[s] : CS0[s] + CSL[s]], yo)
                if dm in (5, 6):
                    k0 = (dm - 5) * (HKO // 2)
                    nc.sync.dma_start(sw2b_sb[:, k0 : k0 + HKO // 2, :],
                                      sw2p[:, k0 : k0 + HKO // 2, 512:1024])

            CHA = [(tt, 0) for tt in range(4)] + [(0, 1), (1, 1), (2, 1)]
            CHB = [(3, 1)]
            pa = [php.tile([P, 512], f32, tag=f"ph{i % 4}", name=f"pya{i}")
                  for i in range(7)]
            for hk in range(HKO):
                swb = sw2pool.tile([P, 512], bf16, tag="swb")
                nc.sync.dma_start(swb, sw2p[:, hk, 0:512])
                for i, (tt, half) in enumerate(CHA):
                    nc.tensor.matmul(pa[i],
                                     hs[:, hk, tt * P : (tt + 1) * P],
                                     swb if half == 0 else sw2b_sb[:, hk, :],
                                     start=(hk == 0), stop=(hk == HKO - 1))
            pb = [php.tile([P, 512], f32, tag=f"ph{3 + i}", name=f"pyb{i}")
                  for i in range(1)]
            for hk in range(HKO):
                for i, (tt, half) in enumerate(CHB):
                    nc.tensor.matmul(pb[i],
                                     hs[:, hk, tt * P : (tt + 1) * P],
                                     sw2b_sb[:, hk, :],
                                     start=(hk == 0), stop=(hk == HKO - 1))
            for i, (tt, half) in enumerate(CHA):
                o = ysop.tile([P, 512], bf16, tag="yso")
                if i % 2 == 0:
                    nc.vector.tensor_copy(o, pa[i])
                else:
                    nc.scalar.activation(o, pa[i], AF.Identity)
                nc.sync.dma_start(
                    ys[tt * P : (tt + 1) * P, half * 512 : (half + 1) * 512], o)
            for i, (tt, half) in enumerate(CHB):
                o = ysop.tile([P, 512], bf16, tag="ysob", name="ysob")
                if i % 2 == 0:
                    nc.vector.tensor_copy(o, pb[i])
                else:
                    nc.scalar.activation(o, pb[i], AF.Identity)
                nc.sync.dma_start(
                    ys[tt * P : (tt + 1) * P, half * 512 : (half + 1) * 512], o)

    nc.finalize()
    return nc


def _get(name, builder):
    if name not in _nc_cache:
        _nc_cache[name] = builder()
    return _nc_cache[name]


def _route_host(x, router_w, router_b):
    X = np.ascontiguousarray(x.reshape(T, DIM)).astype(np.float32)
    logits = X @ router_w.astype(np.float32) + router_b.astype(np.float32)
    lm = logits - logits.max(axis=-1, keepdims=True)
    p = np.exp(lm)
    p /= p.sum(axis=-1, keepdims=True)
    top2 = np.argsort(-logits, axis=-1, kind="stable")[:, :2]
    ct = np.zeros((T, E), np.float32)
    np.put_along_axis(ct, top2, np.take_along_axis(p, top2, axis=-1), axis=-1)
    return ct


def _prep_moe(x, ct, w1, b1, w2, sw1, sb1, sw2):
    xbf = np.ascontiguousarray(x.reshape(T, DIM)).astype(BF16)
    sels = [np.nonzero(ct[:, e])[0] for e in range(E)]
    if max(len(s) for s in sels) > C:
        return None, None
    sw1se = np.ascontiguousarray(
        sw1.reshape(DKO, P, HKO, P).transpose(2, 1, 0, 3)).astype(BF16)
    sb1ce = np.ascontiguousarray(sb1.astype(np.float32).reshape(HKO, P).T)
    sw2pe = np.ascontiguousarray(
        sw2.reshape(HKO, P, DIM).transpose(1, 0, 2)).astype(BF16)
    maps = []
    for e in range(E):
        sel = sels[e]
        n = len(sel)
        selp = np.concatenate([sel, np.zeros(C - n, np.int64)])
        cev = np.zeros(C, np.float32)
        cev[:n] = ct[sel, e]
        g = xbf[selp]
        xgp = np.ascontiguousarray(g.reshape(C, DKO, P).transpose(2, 1, 0))
        blk = xbf[e * BLK : (e + 1) * BLK]
        xbp = np.ascontiguousarray(blk.reshape(BLK, DKO, P).transpose(2, 1, 0))
        w1se = np.ascontiguousarray(
            w1[e].reshape(DKO, P, HKO, P).transpose(2, 1, 0, 3)).astype(BF16)
        b1ce = np.ascontiguousarray(b1[e].astype(np.float32).reshape(HKO, P).T)
        w2se = np.ascontiguousarray(
            w2[e].reshape(HKO, P, DKO, P).transpose(2, 1, 0, 3)).astype(BF16)
        cebe = np.ascontiguousarray(np.broadcast_to(cev[None, :], (P, C)))
        maps.append(dict(
            xg0=np.ascontiguousarray(xgp[:, :, CS0[0] : CS0[0] + CSL[0]]),
            xg1=np.ascontiguousarray(xgp[:, :, CS0[1] : CS0[1] + CSL[1]]),
            xg2=np.ascontiguousarray(xgp[:, :, CS0[2] : CS0[2] + CSL[2]]),
            xb=xbp, ceb=cebe, w1s=w1se, b1c=b1ce, w2s=w2se,
            sw1s=sw1se, sb1c=sb1ce, sw2p=sw2pe))
    return maps, sels


def _run_spmd(nc, in_maps, trace=False):
    from concourse.bass_utils import run_bass_kernel_spmd
    return run_bass_kernel_spmd(nc, in_maps, core_ids=list(range(NCORES)), trace=trace)


def _prep_in_maps(x, router_w, router_b, w1, b1, w2, b2, sw1, sb1, sw2, sb2):
    xt = np.ascontiguousarray(x.reshape(T, DIM).astype(np.float32).T)
    xt32p = np.ascontiguousarray(xt.reshape(DKO, P, T).transpose(1, 0, 2))
    xtbfp = xt32p.astype(BF16)
    rwp = np.ascontiguousarray(
        router_w.astype(np.float32).reshape(DKO, P, E).transpose(1, 0, 2)
    )
    rb = np.tile(router_b.astype(np.float32)[None, :], (P, 1))
    rb = np.ascontiguousarray(rb)
    in_maps = []
    for e in range(NCORES):
        w1p = np.ascontiguousarray(
            w1[e].reshape(DKO, P, H).transpose(1, 0, 2)
        ).astype(BF16)
        b1ce = np.ascontiguousarray(b1[e].astype(np.float32).reshape(HKO, P).T)
        w2bb = np.ascontiguousarray(
            w2[e].reshape(HKO, P, DKO, P).transpose(2, 1, 0, 3)
        ).astype(BF16)
        b2ce = np.ascontiguousarray(b2[e].astype(np.float32).reshape(DKO, P).T)
        s0 = e * SH
        sw1pe = np.ascontiguousarray(
            sw1[:, s0 : s0 + SH].reshape(DKO, P, SH).transpose(1, 0, 2)
        ).astype(BF16)
        sb1ce = np.ascontiguousarray(
            sb1[s0 : s0 + SH].astype(np.float32).reshape(SHKO, P).T
        )
        sw2pe = np.ascontiguousarray(
            sw2[s0 : s0 + SH, :].reshape(SHKO, P, DIM).transpose(1, 0, 2)
        ).astype(BF16)
        sb2v = sb2 if e == 0 else np.zeros_like(sb2)
        sb2ce = np.ascontiguousarray(sb2v.astype(np.float32).reshape(DKO, P).T)
        ohm = np.zeros((E, P), np.float32)
        ohm[e, :] = 1.0
        in_maps.append(
            dict(
                xtbf=xtbfp,
                xt32=xt32p,
                rwp=rwp,
                rb=rb,
                w1p=w1p,
                b1c=b1ce,
                w2b=w2bb,
                b2c=b2ce,
                sw1p=sw1pe,
                sb1c=sb1ce,
                sw2p=sw2pe,
                sb2c=sb2ce,
                oh=ohm,
            )
        )
    return in_maps


def _kernel_dense(x, router_w, router_b, w1, b1, w2, b2, sw1, sb1, sw2, sb2):
    in_maps = _prep_in_maps(x, router_w, router_b, w1, b1, w2, b2, sw1, sb1, sw2, sb2)
    res = _run_spmd(_get("dense", _build_nc), in_maps)
    outT = np.concatenate([r["out"] for r in res.results], axis=0)
    return np.ascontiguousarray(outT.T).reshape(2, 2048, DIM)


_LAST_PATH = [None]


def _kernel_sparse(x, router_w, router_b, w1, b1, w2, b2, sw1, sb1, sw2, sb2):
    ct = _route_host(x, router_w, router_b)
    maps, sels = _prep_moe(x, ct, w1, b1, w2, sw1, sb1, sw2)
    if maps is None:
        _LAST_PATH[0] = "dense"
        return _kernel_dense(x, router_w, router_b, w1, b1, w2, b2,
                             sw1, sb1, sw2, sb2)
    _LAST_PATH[0] = "moe"
    res = _run_spmd(_get("moe", _build_moe), maps)
    out = ct @ b2.astype(np.float32)
    out += sb2.astype(np.float32)[None, :]
    for e in range(E):
        n = len(sels[e])
        yt = res.results[e]["ye"].reshape(DIM, C)[:, :n]
        out[sels[e]] += yt.T.astype(np.float32)
        out[e * BLK : (e + 1) * BLK] += res.results[e]["ys"].astype(np.float32)
    return out.reshape(2, 2048, DIM)


def kernel(x, router_w, router_b, w1, b1, w2, b2, sw1, sb1, sw2, sb2):
    import os
    if os.environ.get("MOE_DENSE"):
        return _kernel_dense(x, router_w, router_b, w1, b1, w2, b2, sw1, sb1, sw2, sb2)
    return _kernel_sparse(x, router_w, router_b, w1, b1, w2, b2, sw1, sb1, sw2, sb2)


# revision 14
# speedup vs baseline: 1.3804x; 1.0039x over previous
import sys

if "/opt/trn_rl_repo" not in sys.path:
    sys.path.insert(0, "/opt/trn_rl_repo")

import numpy as np
import ml_dtypes

DIM = 1024
E = 8
H = 4096
T = 4096
NCORES = 8
P = 128
DKO = DIM // P
HKO = H // P
SH = H // NCORES
SHKO = SH // P
SLICE = 512
NSLICES = T // SLICE

C = 1091
CSL = [512, 512, C - 1024]
CS0 = [0, 512, 1024]
BLK = T // NCORES

BF16 = ml_dtypes.bfloat16

_nc_cache = {}


def _build_nc(sim=False):
    import concourse.mybir as mybir
    import concourse.tile as tile
    from concourse import bacc
    from concourse.masks import make_identity

    f32 = mybir.dt.float32
    bf16 = mybir.dt.bfloat16
    AF = mybir.ActivationFunctionType
    OP = mybir.AluOpType
    AX = mybir.AxisListType

    ndev = 1 if sim else NCORES
    nc = bacc.Bacc("TRN2", target_bir_lowering=False, debug=False, num_devices=ndev)

    xtbf = nc.dram_tensor("xtbf", [P, DKO, T], bf16, kind="ExternalInput")
    xt32 = nc.dram_tensor("xt32", [P, DKO, T], f32, kind="ExternalInput")
    rwp = nc.dram_tensor("rwp", [P, DKO, E], f32, kind="ExternalInput")
    rb = nc.dram_tensor("rb", [P, E], f32, kind="ExternalInput")
    w1p = nc.dram_tensor("w1p", [P, DKO, H], bf16, kind="ExternalInput")
    b1c = nc.dram_tensor("b1c", [P, HKO], f32, kind="ExternalInput")
    w2b = nc.dram_tensor("w2b", [DKO, P, HKO, P], bf16, kind="ExternalInput")
    b2c = nc.dram_tensor("b2c", [P, DKO], f32, kind="ExternalInput")
    sw1p = nc.dram_tensor("sw1p", [P, DKO, SH], bf16, kind="ExternalInput")
    sb1c = nc.dram_tensor("sb1c", [P, SHKO], f32, kind="ExternalInput")
    sw2p = nc.dram_tensor("sw2p", [P, SHKO, DIM], bf16, kind="ExternalInput")
    sb2c = nc.dram_tensor("sb2c", [P, DKO], f32, kind="ExternalInput")
    oh = nc.dram_tensor("oh", [E, P], f32, kind="ExternalInput")
    out = nc.dram_tensor("out", [P, T], f32, kind="ExternalOutput")

    with tile.TileContext(nc) as tc:
        with (
            tc.tile_pool(name="const", bufs=1) as const,
            tc.tile_pool(name="wpool", bufs=1) as wpool,
        ):
            ident = const.tile([P, P], f32)
            make_identity(nc, ident)
            rwp_sb = const.tile([P, DKO, E], f32)
            nc.sync.dma_start(rwp_sb, rwp[:, :, :])
            rb_sb = const.tile([P, E], f32)
            nc.sync.dma_start(rb_sb, rb[:, :])
            b1c_sb = const.tile([P, HKO], f32)
            nc.sync.dma_start(b1c_sb, b1c[:, :])
            b2c_sb = const.tile([P, DKO], f32)
            nc.sync.dma_start(b2c_sb, b2c[:, :])
            sb1c_sb = const.tile([P, SHKO], f32)
            nc.sync.dma_start(sb1c_sb, sb1c[:, :])
            sb2c_sb = const.tile([P, DKO], f32)
            nc.sync.dma_start(sb2c_sb, sb2c[:, :])
            oh_sb = const.tile([E, P], f32)
            nc.sync.dma_start(oh_sb, oh[:, :])

            w1_sb = wpool.tile([P, DKO, H], bf16)
            nc.sync.dma_start(w1_sb, w1p[:, :, :])
            sw1_sb = wpool.tile([P, DKO, SH], bf16)
            nc.sync.dma_start(sw1_sb, sw1p[:, :, :])
            sw2_sb = wpool.tile([P, SHKO, DIM], bf16)
            nc.sync.dma_start(sw2_sb, sw2p[:, :, :])
            ct_sb = wpool.tile([E, T], f32)

            with (
                tc.tile_pool(name="rx", bufs=2) as rxp,
                tc.tile_pool(name="rt", bufs=2) as rt,
                tc.tile_pool(name="rps", bufs=2, space="PSUM") as rps,
                tc.tile_pool(name="tps", bufs=2, space="PSUM") as tps,
            ):
                for tt in range(T // P):
                    rx = rxp.tile([P, DKO, P], f32, tag="rx")
                    nc.sync.dma_start(rx, xt32[:, :, tt * P : (tt + 1) * P])
                    pl = rps.tile([P, E], f32, tag="pl")
                    for ko in range(DKO):
                        nc.tensor.matmul(
                            pl,
                            rx[:, ko, :],
                            rwp_sb[:, ko, :],
                            start=(ko == 0),
                            stop=(ko == DKO - 1),
                        )
                    lg = rt.tile([P, E], f32, tag="lg")
                    nc.vector.tensor_add(lg, pl, rb_sb)
                    mx = rt.tile([P, 1], f32, tag="mx")
                    nc.vector.reduce_max(mx, lg, axis=AX.X)
                    nmx = rt.tile([P, 1], f32, tag="nmx")
                    nc.vector.tensor_scalar_mul(nmx, mx, -1.0)
                    ex = rt.tile([P, E], f32, tag="ex")
                    nc.scalar.activation(ex, lg, AF.Exp, bias=nmx)
                    sm = rt.tile([P, 1], f32, tag="sm")
                    nc.vector.reduce_sum(sm, ex, axis=AX.X)
                    rc = rt.tile([P, 1], f32, tag="rc")
                    nc.vector.reciprocal(rc, sm)
                    ge1 = rt.tile([P, E], f32, tag="ge1")
                    nc.vector.tensor_tensor(ge1, lg, mx.to_broadcast((P, E)), OP.is_ge)
                    big = rt.tile([P, E], f32, tag="big")
                    nc.vector.tensor_scalar_mul(big, ge1, 1e30)
                    lm = rt.tile([P, E], f32, tag="lm")
                    nc.vector.tensor_sub(lm, lg, big)
                    m2 = rt.tile([P, 1], f32, tag="m2")
                    nc.vector.reduce_max(m2, lm, axis=AX.X)
                    msk = rt.tile([P, E], f32, tag="msk")
                    nc.vector.tensor_tensor(msk, lg, m2.to_broadcast((P, E)), OP.is_ge)
                    pw = rt.tile([P, E], f32, tag="pw")
                    nc.vector.tensor_mul(pw, ex, msk)
                    nc.vector.tensor_tensor(pw, pw, rc.to_broadcast((P, E)), OP.mult)
                    cps = tps.tile([E, P], f32, tag="cps")
                    nc.tensor.transpose(cps, pw, ident)
                    nc.vector.tensor_copy(ct_sb[:, tt * P : (tt + 1) * P], cps)

            with (
                tc.tile_pool(name="xp", bufs=2) as xp,
                tc.tile_pool(name="w2p", bufs=2) as w2p,
                tc.tile_pool(name="hp", bufs=1) as hp,
                tc.tile_pool(name="cep", bufs=1) as cep,
                tc.tile_pool(name="op", bufs=1) as op_,
                tc.tile_pool(name="tp", bufs=1) as tp_,
                tc.tile_pool(name="dram", bufs=1, space="DRAM") as dram,
                tc.tile_pool(name="p1", bufs=2, space="PSUM") as p1p,
                tc.tile_pool(name="p2", bufs=2, space="PSUM") as p2p,
                tc.tile_pool(name="p2s", bufs=1, space="PSUM") as p2sp,
                tc.tile_pool(name="pc", bufs=1, space="PSUM") as pcp,
            ):
                for s in range(NSLICES):
                    t0 = s * SLICE
                    xt = xp.tile([P, DKO, SLICE], bf16, tag="xt")
                    nc.sync.dma_start(xt, xtbf[:, :, t0 : t0 + SLICE])
                    cei = pcp.tile([P, SLICE], f32, tag="cei")
                    nc.tensor.matmul(
                        cei, oh_sb, ct_sb[:, t0 : t0 + SLICE], start=True, stop=True
                    )
                    ce = cep.tile([P, SLICE], f32, tag="ce")
                    nc.vector.tensor_copy(ce, cei)

                    h = hp.tile([P, HKO + SHKO, SLICE], bf16, tag="h")
                    for hm in range(HKO):
                        ps = p1p.tile([P, SLICE], f32, tag="ps1")
                        for ko in range(DKO):
                            nc.tensor.matmul(
                                ps,
                                w1_sb[:, ko, hm * P : (hm + 1) * P],
                                xt[:, ko, :],
                                start=(ko == 0),
                                stop=(ko == DKO - 1),
                            )
                        nc.scalar.activation(
                            h[:, hm, :], ps, AF.Gelu, bias=b1c_sb[:, hm : hm + 1]
                        )
                    for sm_ in range(SHKO):
                        ps = p1p.tile([P, SLICE], f32, tag="ps1")
                        for ko in range(DKO):
                            nc.tensor.matmul(
                                ps,
                                sw1_sb[:, ko, sm_ * P : (sm_ + 1) * P],
                                xt[:, ko, :],
                                start=(ko == 0),
                                stop=(ko == DKO - 1),
                            )
                        nc.scalar.activation(
                            h[:, HKO + sm_, :], ps, AF.Gelu, bias=sb1c_sb[:, sm_ : sm_ + 1]
                        )

                    ob = op_.tile([P, DKO, SLICE], f32, tag="ob")
                    for dm in range(DKO):
                        w2t = w2p.tile([P, HKO, P], bf16, tag="w2t")
                        nc.sync.dma_start(w2t, w2b[dm, :, :, :])
                        ps2 = p2p.tile([P, SLICE], f32, tag="ps2")
                        for hk in range(HKO):
                            nc.tensor.matmul(
                                ps2,
                                w2t[:, hk, :],
                                h[:, hk, :],
                                start=(hk == 0),
                                stop=(hk == HKO - 1),
                            )
                        ps2s = p2sp.tile([P, SLICE], f32, tag="ps2s")
                        for sk in range(SHKO):
                            nc.tensor.matmul(
                                ps2s,
                                sw2_sb[:, sk, dm * P : (dm + 1) * P],
                                h[:, HKO + sk, :],
                                start=(sk == 0),
                                stop=(sk == SHKO - 1),
                            )
                        t1 = tp_.tile([P, SLICE], f32, tag="t1")
                        nc.scalar.activation(
                            t1, ps2, AF.Identity, bias=b2c_sb[:, dm : dm + 1]
                        )
                        nc.vector.tensor_mul(t1, t1, ce)
                        t2 = tp_.tile([P, SLICE], f32, tag="t2")
                        nc.scalar.activation(
                            t2, ps2s, AF.Identity, bias=sb2c_sb[:, dm : dm + 1]
                        )
                        nc.vector.tensor_add(ob[:, dm, :], t1, t2)

                    obd = dram.tile([DIM, SLICE], f32, tag=f"obd{s}", name=f"obd{s}")
                    nc.sync.dma_start(
                        obd.rearrange("(dm ki) t -> ki dm t", ki=P), ob
                    )
                    rso = dram.tile([P, SLICE], f32, tag=f"rso{s}", name=f"rso{s}")
                    if sim:
                        nc.sync.dma_start(rso[:, :], obd[0:P, :])
                    else:
                        nc.gpsimd.collective_compute(
                            "ReduceScatter",
                            OP.add,
                            replica_groups=[list(range(NCORES))],
                            ins=[obd.opt()],
                            outs=[rso.opt()],
                        )
                    nc.sync.dma_start(out[:, t0 : t0 + SLICE], rso[:, :])

    nc.finalize()
    return nc


def _build_moe(sim=False):
    import concourse.mybir as mybir
    import concourse.tile as tile
    from concourse import bacc

    f32 = mybir.dt.float32
    bf16 = mybir.dt.bfloat16
    AF = mybir.ActivationFunctionType
    OP = mybir.AluOpType

    nc = bacc.Bacc("TRN2", target_bir_lowering=False, debug=False,
                   num_devices=1 if sim else NCORES)

    xg0 = nc.dram_tensor("xg0", [P, DKO, CSL[0]], bf16, kind="ExternalInput")
    xg1 = nc.dram_tensor("xg1", [P, DKO, CSL[1]], bf16, kind="ExternalInput")
    xg2 = nc.dram_tensor("xg2", [P, DKO, CSL[2]], bf16, kind="ExternalInput")
    xb = nc.dram_tensor("xb", [P, DKO, BLK], bf16, kind="ExternalInput")
    ceb = nc.dram_tensor("ceb", [P, C], f32, kind="ExternalInput")
    w1s = nc.dram_tensor("w1s", [HKO, P, DKO, P], bf16, kind="ExternalInput")
    b1c = nc.dram_tensor("b1c", [P, HKO], f32, kind="ExternalInput")
    w2s = nc.dram_tensor("w2s", [DKO, P, HKO, P], bf16, kind="ExternalInput")
    sw1s = nc.dram_tensor("sw1s", [HKO, P, DKO, P], bf16, kind="ExternalInput")
    sb1c = nc.dram_tensor("sb1c", [P, HKO], f32, kind="ExternalInput")
    sw2p = nc.dram_tensor("sw2p", [P, HKO, DIM], bf16, kind="ExternalInput")
    ye = nc.dram_tensor("ye", [DKO, P, C], bf16, kind="ExternalOutput")
    ys = nc.dram_tensor("ys", [BLK, DIM], bf16, kind="ExternalOutput")

    with tile.TileContext(nc) as tc:
        with (
            tc.tile_pool(name="const", bufs=1) as const,
            tc.tile_pool(name="xgp", bufs=1) as xgp,
            tc.tile_pool(name="hbuf", bufs=1) as hbuf,
            tc.tile_pool(name="w1p", bufs=4) as w1p,
            tc.tile_pool(name="w2p", bufs=2) as w2p,
            tc.tile_pool(name="sw1p", bufs=4) as sw1p,
            tc.tile_pool(name="sw2pool", bufs=4) as sw2pool,
            tc.tile_pool(name="yop", bufs=2) as yop,
            tc.tile_pool(name="ysop", bufs=2) as ysop,
            tc.tile_pool(name="warm", bufs=1) as warm,
            tc.tile_pool(name="ph", bufs=2, space="PSUM") as php,
        ):
            wza = warm.tile([P, P], bf16)
            nc.vector.memset(wza, 0.0)
            wzb = warm.tile([P, 512], bf16)
            nc.vector.memset(wzb, 0.0)

            swblk0 = sw1p.tile([P, DKO, P], bf16, tag="swblk", name="swblk0")
            nc.sync.dma_start(swblk0, sw1s[0, :, :, :])
            xb_sb = xgp.tile([P, DKO, BLK], bf16, name="xb_sb")
            nc.sync.dma_start(xb_sb[:, 0 : DKO // 2, :], xb[:, 0 : DKO // 2, :])
            sb1c_sb = const.tile([P, HKO], f32)
            nc.sync.dma_start(sb1c_sb, sb1c[:, :])
            nc.sync.dma_start(xb_sb[:, DKO // 2 : DKO, :], xb[:, DKO // 2 : DKO, :])

            h = hbuf.tile([P, HKO, C], bf16)
            hs = hbuf.tile([P, HKO, BLK], bf16)
            sw2b_sb = hbuf.tile([P, HKO, 512], bf16)
            xgs = [xgp.tile([P, DKO, CSL[i]], bf16, name=f"xg_sb{i}")
                   for i in range(3)]
            b1c_sb = const.tile([P, HKO], f32)
            ce_sb = const.tile([P, C], f32)

            wps = php.tile([P, 512], f32, tag="ph3", name="wps")
            for k in range(10):
                nc.tensor.matmul(wps, wza, wzb,
                                 start=(k == 0), stop=(k == 9))

            for hm in range(HKO):
                if hm == 0:
                    swblk = swblk0
                else:
                    swblk = sw1p.tile([P, DKO, P], bf16, tag="swblk")
                    nc.sync.dma_start(swblk, sw1s[hm, :, :, :])
                ps = php.tile([P, BLK], f32, tag="ph0", name="psh")
                for ko in range(DKO):
                    nc.tensor.matmul(ps, swblk[:, ko, :], xb_sb[:, ko, :],
                                     start=(ko == 0), stop=(ko == DKO - 1))
                nc.scalar.activation(hs[:, hm, :], ps, AF.Gelu,
                                     bias=sb1c_sb[:, hm : hm + 1])
                if hm == 6:
                    nc.sync.dma_start(xgs[0], xg0[:, :, :])
                elif hm == 11:
                    nc.sync.dma_start(xgs[1], xg1[:, :, :])
                elif hm == 16:
                    nc.sync.dma_start(xgs[2], xg2[:, :, :])
                elif hm == 21:
                    nc.sync.dma_start(b1c_sb, b1c[:, :])
                    nc.sync.dma_start(ce_sb, ceb[:, :])

            for hm in range(HKO):
                w1blk = w1p.tile([P, DKO, P], bf16, tag="w1blk")
                nc.sync.dma_start(w1blk, w1s[hm, :, :, :])
                pss = [php.tile([P, 512], f32, tag=f"ph{s}", name=f"ph{s}")
                       for s in range(3)]
                for ko in range(DKO):
                    for s in range(3):
                        nc.tensor.matmul(pss[s][:, : CSL[s]],
                                         w1blk[:, ko, :],
                                         xgs[s][:, ko, :],
                                         start=(ko == 0), stop=(ko == DKO - 1))
                for s in range(3):
                    nc.scalar.activation(h[:, hm, CS0[s] : CS0[s] + CSL[s]],
                                         pss[s][:, : CSL[s]], AF.Gelu,
                                         bias=b1c_sb[:, hm : hm + 1])
                    nc.vector.tensor_tensor(h[:, hm, CS0[s] : CS0[s] + CSL[s]],
                                            h[:, hm, CS0[s] : CS0[s] + CSL[s]],
                                            ce_sb[:, CS0[s] : CS0[s] + CSL[s]],
                                            OP.mult)
                if hm in (8, 16):
                    k0 = 0 if hm == 8 else HKO // 2
                    nc.sync.dma_start(sw2b_sb[:, k0 : k0 + HKO // 2, :],
                                      sw2p[:, k0 : k0 + HKO // 2, 512:1024])

            CHA = [(tt, 0) for tt in range(4)] + [(0, 1), (1, 1), (2, 1)]
            CHB = [(3, 1)]
            pa = [php.tile([P, 512], f32, tag=f"ph{i % 4}", name=f"pya{i}")
                  for i in range(7)]
            w2pre = []
            for hk in range(HKO):
                swb = sw2pool.tile([P, 512], bf16, tag="swb")
                nc.sync.dma_start(swb, sw2p[:, hk, 0:512])
                if hk in (8, 20):
                    w2blk = w2p.tile([P, HKO, P], bf16, tag="w2blk",
                                     name="w2pre")
                    nc.sync.dma_start(w2blk, w2s[len(w2pre), :, :, :])
                    w2pre.append(w2blk)
                for i, (tt, half) in enumerate(CHA):
                    nc.tensor.matmul(pa[i],
                                     hs[:, hk, tt * P : (tt + 1) * P],
                                     swb if half == 0 else sw2b_sb[:, hk, :],
                                     start=(hk == 0), stop=(hk == HKO - 1))
            pb = [php.tile([P, 512], f32, tag=f"ph{3 + i}", name=f"pyb{i}")
                  for i in range(1)]
            for hk in range(HKO):
                for i, (tt, half) in enumerate(CHB):
                    nc.tensor.matmul(pb[i],
                                     hs[:, hk, tt * P : (tt + 1) * P],
                                     sw2b_sb[:, hk, :],
                                     start=(hk == 0), stop=(hk == HKO - 1))
            for i, (tt, half) in enumerate(CHA):
                o = ysop.tile([P, 512], bf16, tag="yso")
                if i % 2 == 0:
                    nc.vector.tensor_copy(o, pa[i])
                else:
                    nc.scalar.activation(o, pa[i], AF.Identity)
                nc.sync.dma_start(
                    ys[tt * P : (tt + 1) * P, half * 512 : (half + 1) * 512], o)
            for i, (tt, half) in enumerate(CHB):
                o = ysop.tile([P, 512], bf16, tag="ysob", name="ysob")
                if i % 2 == 0:
                    nc.vector.tensor_copy(o, pb[i])
                else:
                    nc.scalar.activation(o, pb[i], AF.Identity)
                nc.sync.dma_start(
                    ys[tt * P : (tt + 1) * P, half * 512 : (half + 1) * 512], o)
            for dm in range(DKO):
                if dm < 2:
                    w2blk = w2pre[dm]
                else:
                    w2blk = w2p.tile([P, HKO, P], bf16, tag="w2blk")
                    nc.sync.dma_start(w2blk, w2s[dm, :, :, :])
                pss = [php.tile([P, 512], f32, tag=f"ph{s}", name=f"ph{s}")
                       for s in range(3)]
                for hk in range(HKO):
                    for s in range(3):
                        nc.tensor.matmul(pss[s][:, : CSL[s]],
                                         w2blk[:, hk, :],
                                         h[:, hk, CS0[s] : CS0[s] + CSL[s]],
                                         start=(hk == 0), stop=(hk == HKO - 1))
                for s in range(3):
                    yo = yop.tile([P, CSL[s]], bf16, tag=f"yo{s}")
                    if s == 1:
                        nc.scalar.activation(yo, pss[s][:, : CSL[s]],
                                             AF.Identity)
                    else:
                        nc.vector.tensor_copy(yo, pss[s][:, : CSL[s]])
                    nc.sync.dma_start(ye[dm, :, CS0[s] : CS0[s] + CSL[s]], yo)
    nc.finalize()
    return nc


def _get(name, builder):
    if name not in _nc_cache:
        _nc_cache[name] = builder()
    return _nc_cache[name]


def _route_host(x, router_w, router_b):
    X = np.ascontiguousarray(x.reshape(T, DIM)).astype(np.float32)
    logits = X @ router_w.astype(np.float32) + router_b.astype(np.float32)
    lm = logits - logits.max(axis=-1, keepdims=True)
    p = np.exp(lm)
    p /= p.sum(axis=-1, keepdims=True)
    top2 = np.argsort(-logits, axis=-1, kind="stable")[:, :2]
    ct = np.zeros((T, E), np.float32)
    np.put_along_axis(ct, top2, np.take_along_axis(p, top2, axis=-1), axis=-1)
    return ct


def _prep_moe(x, ct, w1, b1, w2, sw1, sb1, sw2):
    xbf = np.ascontiguousarray(x.reshape(T, DIM)).astype(BF16)
    sels = [np.nonzero(ct[:, e])[0] for e in range(E)]
    if max(len(s) for s in sels) > C:
        return None, None
    sw1se = np.ascontiguousarray(
        sw1.reshape(DKO, P, HKO, P).transpose(2, 1, 0, 3)).astype(BF16)
    sb1ce = np.ascontiguousarray(sb1.astype(np.float32).reshape(HKO, P).T)
    sw2pe = np.ascontiguousarray(
        sw2.reshape(HKO, P, DIM).transpose(1, 0, 2)).astype(BF16)
    maps = []
    for e in range(E):
        sel = sels[e]
        n = len(sel)
        selp = np.concatenate([sel, np.zeros(C - n, np.int64)])
        cev = np.zeros(C, np.float32)
        cev[:n] = ct[sel, e]
        g = xbf[selp]
        xgp = np.ascontiguousarray(g.reshape(C, DKO, P).transpose(2, 1, 0))
        blk = xbf[e * BLK : (e + 1) * BLK]
        xbp = np.ascontiguousarray(blk.reshape(BLK, DKO, P).transpose(2, 1, 0))
        w1se = np.ascontiguousarray(
            w1[e].reshape(DKO, P, HKO, P).transpose(2, 1, 0, 3)).astype(BF16)
        b1ce = np.ascontiguousarray(b1[e].astype(np.float32).reshape(HKO, P).T)
        w2se = np.ascontiguousarray(
            w2[e].reshape(HKO, P, DKO, P).transpose(2, 1, 0, 3)).astype(BF16)
        cebe = np.ascontiguousarray(np.broadcast_to(cev[None, :], (P, C)))
        maps.append(dict(
            xg0=np.ascontiguousarray(xgp[:, :, CS0[0] : CS0[0] + CSL[0]]),
            xg1=np.ascontiguousarray(xgp[:, :, CS0[1] : CS0[1] + CSL[1]]),
            xg2=np.ascontiguousarray(xgp[:, :, CS0[2] : CS0[2] + CSL[2]]),
            xb=xbp, ceb=cebe, w1s=w1se, b1c=b1ce, w2s=w2se,
            sw1s=sw1se, sb1c=sb1ce, sw2p=sw2pe))
    return maps, sels


def _run_spmd(nc, in_maps, trace=False):
    from concourse.bass_utils import run_bass_kernel_spmd
    return run_bass_kernel_spmd(nc, in_maps, core_ids=list(range(NCORES)), trace=trace)


def _prep_in_maps(x, router_w, router_b, w1, b1, w2, b2, sw1, sb1, sw2, sb2):
    xt = np.ascontiguousarray(x.reshape(T, DIM).astype(np.float32).T)
    xt32p = np.ascontiguousarray(xt.reshape(DKO, P, T).transpose(1, 0, 2))
    xtbfp = xt32p.astype(BF16)
    rwp = np.ascontiguousarray(
        router_w.astype(np.float32).reshape(DKO, P, E).transpose(1, 0, 2)
    )
    rb = np.tile(router_b.astype(np.float32)[None, :], (P, 1))
    rb = np.ascontiguousarray(rb)
    in_maps = []
    for e in range(NCORES):
        w1p = np.ascontiguousarray(
            w1[e].reshape(DKO, P, H).transpose(1, 0, 2)
        ).astype(BF16)
        b1ce = np.ascontiguousarray(b1[e].astype(np.float32).reshape(HKO, P).T)
        w2bb = np.ascontiguousarray(
            w2[e].reshape(HKO, P, DKO, P).transpose(2, 1, 0, 3)
        ).astype(BF16)
        b2ce = np.ascontiguousarray(b2[e].astype(np.float32).reshape(DKO, P).T)
        s0 = e * SH
        sw1pe = np.ascontiguousarray(
            sw1[:, s0 : s0 + SH].reshape(DKO, P, SH).transpose(1, 0, 2)
        ).astype(BF16)
        sb1ce = np.ascontiguousarray(
            sb1[s0 : s0 + SH].astype(np.float32).reshape(SHKO, P).T
        )
        sw2pe = np.ascontiguousarray(
            sw2[s0 : s0 + SH, :].reshape(SHKO, P, DIM).transpose(1, 0, 2)
        ).astype(BF16)
        sb2v = sb2 if e == 0 else np.zeros_like(sb2)
        sb2ce = np.ascontiguousarray(sb2v.astype(np.float32).reshape(DKO, P).T)
        ohm = np.zeros((E, P), np.float32)
        ohm[e, :] = 1.0
        in_maps.append(
            dict(
                xtbf=xtbfp,
                xt32=xt32p,
                rwp=rwp,
                rb=rb,
                w1p=w1p,
                b1c=b1ce,
                w2b=w2bb,
                b2c=b2ce,
                sw1p=sw1pe,
                sb1c=sb1ce,
                sw2p=sw2pe,
                sb2c=sb2ce,
                oh=ohm,
            )
        )
    return in_maps


def _kernel_dense(x, router_w, router_b, w1, b1, w2, b2, sw1, sb1, sw2, sb2):
    in_maps = _prep_in_maps(x, router_w, router_b, w1, b1, w2, b2, sw1, sb1, sw2, sb2)
    res = _run_spmd(_get("dense", _build_nc), in_maps)
    outT = np.concatenate([r["out"] for r in res.results], axis=0)
    return np.ascontiguousarray(outT.T).reshape(2, 2048, DIM)


_LAST_PATH = [None]


def _kernel_sparse(x, router_w, router_b, w1, b1, w2, b2, sw1, sb1, sw2, sb2):
    ct = _route_host(x, router_w, router_b)
    maps, sels = _prep_moe(x, ct, w1, b1, w2, sw1, sb1, sw2)
    if maps is None:
        _LAST_PATH[0] = "dense"
        return _kernel_dense(x, router_w, router_b, w1, b1, w2, b2,
                             sw1, sb1, sw2, sb2)
    _LAST_PATH[0] = "moe"
    res = _run_spmd(_get("moe", _build_moe), maps)
    out = ct @ b2.astype(np.float32)
    out += sb2.astype(np.float32)[None, :]
    for e in range(E):
        n = len(sels[e])
        yt = res.results[e]["ye"].reshape(DIM, C)[:, :n]
        out[sels[e]] += yt.T.astype(np.float32)
        out[e * BLK : (e + 1) * BLK] += res.results[e]["ys"].astype(np.float32)
    return out.reshape(2, 2048, DIM)


def kernel(x, router_w, router_b, w1, b1, w2, b2, sw1, sb1, sw2, sb2):
    import os
    if os.environ.get("MOE_DENSE"):
        return _kernel_dense(x, router_w, router_b, w1, b1, w2, b2, sw1, sb1, sw2, sb2)
    return _kernel_sparse(x, router_w, router_b, w1, b1, w2, b2, sw1, sb1, sw2, sb2)
